# revision 1
# baseline (speedup 1.0000x reference)
"""GCN (2-layer GCNConv + global mean pool + linear head) on 8 Trainium2 cores.

Strategy (follows the sharding hint):
  - Nodes sharded contiguously: core k owns nodes [6272k, 6272k+6272).
  - Edges sharded by destination; per dst-block (112 nodes) edge lists are
    built host-side (index-only preprocessing) and padded to uniform size.
  - GCNConv: out = dis * (sum_{e:dst=v} tab[src_e] + tab[v]) + b with
    tab = (h @ W) * dis (bf16 table), dis = 1/sqrt(deg+1).
  - Neighbor rows are fetched with gpsimd.dma_gather (two <=32768-row table
    halves, int16 indices, 1024 idx/op), summed per dst block with a one-hot
    matmul on the PE (segment-sum), epilogue on DVE/ACT.
  - Per-layer tables are shard-computed then AllGather'd (halo exchange).
  - Pooling: one-hot (node->graph) matmuls -> sums^T/counts, AllReduce,
    mean + linear head on-device.
All numeric compute runs on-device; the host only partitions/reorders indices.
"""
import sys
import types

sys.path.insert(0, "/opt/trn_rl_repo")


def _install_ntff_hook():
    if "antenv.axon_hooks" in sys.modules:
        return
    mod = types.ModuleType("antenv.axon_hooks")
    mod._hook = None
    mod.set_axon_ntff_profile_hook = lambda h: setattr(mod, "_hook", h)
    mod.get_axon_ntff_profile_hook = lambda: mod._hook
    sys.modules["antenv.axon_hooks"] = mod
    sys.path.insert(0, "/root/.axon_site")
    try:
        from trn_agent_boot.trn_boot import _ntff_profile_via_ctypes
        mod.set_axon_ntff_profile_hook(
            _ntff_profile_via_ctypes("/opt/axon/libaxon_pjrt.so"))
    except Exception:
        pass


_install_ntff_hook()

import numpy as np
import ml_dtypes
import concourse.bass as bass
import concourse.bacc as bacc
import concourse.mybir as mybir
import concourse.tile as tile
from concourse import bass_utils
from concourse.masks import make_identity

BF16 = ml_dtypes.bfloat16
P = 128
N = 50000
E = 800000
H = 128
C = 10
G = 512
NCORES = 8
BS = 112                 # nodes per dst block
BPC = 56                 # dst blocks per core
NPC = BPC * BS           # nodes per core (6272)
NPAD = NCORES * NPC      # padded node count (50176)
NHALF = NPAD // 2        # table half rows (25088)
CH = 8                   # chunks per half per block (1024 idx = one dma_gather)
CPB = 2 * CH             # chunks per block
SPB = CPB * P            # slots per block (2048)
FW = CH * P // 16        # idx columns per half per block (64)
NQ = 4

_cache = {}


def _wrap_idx(idx_flat):
    """dma_gather idx layout: position i -> [i%16, i//16], replicated x8."""
    n = idx_flat.shape[0]
    arr = np.ascontiguousarray(idx_flat.reshape(n // 16, 16).T).astype(np.int16)
    return np.tile(arr, (8, 1))


def _prep(x, edge_index, batch, W1, b1, W2, b2, Wl, bl):
    src = np.asarray(edge_index[0], np.int64)
    dst = np.asarray(edge_index[1], np.int64)
    batch = np.asarray(batch, np.int64)
    x = np.asarray(x, np.float32)

    deg = np.bincount(dst, minlength=N).astype(np.float32) + 1.0
    deg_pad = np.ones(NPAD, np.float32)
    deg_pad[:N] = deg

    core_of = dst // NPC
    blk_of = (dst % NPC) // BS

    idxlo = np.zeros((NCORES, P, BPC * FW), np.int16)
    idxhi = np.zeros((NCORES, P, BPC * FW), np.int16)
    dstrel = np.full((NCORES, P, BPC * CPB), -1.0, np.float32)
    for k in range(NCORES):
        m = core_of == k
        sk, dk, bk = src[m], dst[m], blk_of[m]
        order = np.argsort(bk, kind="stable")
        sk, dk, bk = sk[order], dk[order], bk[order]
        bounds = np.searchsorted(bk, np.arange(BPC + 1))
        for b in range(BPC):
            s = dk_s = sk[bounds[b]:bounds[b + 1]]
            rel = (dk[bounds[b]:bounds[b + 1]] - (k * NPC + b * BS)).astype(np.int64)
            lo = s < NHALF
            slo, rlo = s[lo], rel[lo]
            shi, rhi = s[~lo] - NHALF, rel[~lo]
            assert len(slo) <= CH * P and len(shi) <= CH * P, (len(slo), len(shi))
            lo_full = np.zeros(CH * P, np.int64)
            lo_full[:len(slo)] = slo
            hi_full = np.zeros(CH * P, np.int64)
            hi_full[:len(shi)] = shi
            idxlo[k, :, b * FW:(b + 1) * FW] = _wrap_idx(lo_full)
            idxhi[k, :, b * FW:(b + 1) * FW] = _wrap_idx(hi_full)
            for (rels, coff) in ((rlo, 0), (rhi, CH)):
                nr = len(rels)
                ch = np.arange(nr) // P + coff
                pp = np.arange(nr) % P
                dstrel[k, pp, b * CPB + ch] = rels

    # xT tiles per core: [BPC, 128 feat, BS nodes] (host layout transform)
    x_pad = np.zeros((NPAD, P), np.float32)
    x_pad[:N] = x
    xt = np.ascontiguousarray(
        x_pad.reshape(NCORES, BPC, BS, P).transpose(0, 1, 3, 2))

    degc = np.ones((NCORES, P, BPC), np.float32)
    batf = np.full((NCORES, P, BPC), -1.0, np.float32)
    bat_pad = np.full(NPAD, -1.0, np.float32)
    bat_pad[:N] = batch.astype(np.float32)
    for k in range(NCORES):
        degc[k, :BS] = deg_pad[k * NPC:(k + 1) * NPC].reshape(BPC, BS).T
        batf[k, :BS] = bat_pad[k * NPC:(k + 1) * NPC].reshape(BPC, BS).T

    iota_rep = np.tile(np.arange(BS, dtype=np.float32), (P, CPB))
    iota512 = np.tile(np.arange(G, dtype=np.float32), (P, 1))

    common = {
        "W1f": np.asarray(W1, np.float32),
        "W2f": np.asarray(W2, np.float32),
        "Wlf": np.asarray(Wl, np.float32),
        "b1r": np.tile(np.asarray(b1, np.float32), (P, 1)),
        "b2r": np.tile(np.asarray(b2, np.float32), (P, 1)),
        "blc": np.asarray(bl, np.float32).reshape(C, 1),
        "iota_rep": iota_rep,
        "iota512": np.ascontiguousarray(iota512, np.float32),
    }
    in_maps = []
    for k in range(NCORES):
        m = dict(common)
        m["xt"] = xt[k]
        m["idxlo"] = idxlo[k]
        m["idxhi"] = idxhi[k]
        m["dstrel"] = dstrel[k]
        m["degc"] = degc[k]
        m["batf"] = batf[k]
        in_maps.append(m)
    return in_maps


def _build():
    RG = [list(range(NCORES))]
    f32, bf16 = mybir.dt.float32, mybir.dt.bfloat16

    nc = bacc.Bacc("TRN2", target_bir_lowering=False, debug=False,
                   num_devices=NCORES, num_swdge_queues=NQ)

    def inp(name, shape, dt):
        return nc.dram_tensor(name, shape, dt, kind="ExternalInput").ap()

    xt = inp("xt", (BPC, P, BS), f32)
    idxlo = inp("idxlo", (P, BPC * FW), mybir.dt.int16)
    idxhi = inp("idxhi", (P, BPC * FW), mybir.dt.int16)
    dstrel = inp("dstrel", (P, BPC * CPB), f32)
    degc = inp("degc", (P, BPC), f32)
    batf = inp("batf", (P, BPC), f32)
    W1f = inp("W1f", (P, H), f32)
    W2f = inp("W2f", (P, H), f32)
    Wlf = inp("Wlf", (H, C), f32)
    b1r = inp("b1r", (P, H), f32)
    b2r = inp("b2r", (P, H), f32)
    blc = inp("blc", (C, 1), f32)
    iota_rep = inp("iota_rep", (P, CPB * BS), f32)
    iota512 = inp("iota512", (P, G), f32)
    out = nc.dram_tensor("out", (G, C), f32, kind="ExternalOutput").ap()

    with tile.TileContext(nc) as tc:
        with tc.tile_pool(name="const", bufs=1) as cpool, \
             tc.tile_pool(name="dram", bufs=1, space="DRAM") as dpool, \
             tc.tile_pool(name="wtile", bufs=3) as wpool, \
             tc.tile_pool(name="gath", bufs=4) as gpool, \
             tc.tile_pool(name="oh", bufs=4) as ohpool, \
             tc.tile_pool(name="ep", bufs=3) as eppool, \
             tc.tile_pool(name="persist", bufs=1) as ppool:

            tab_shard = [dpool.tile([NPC, H], bf16, tag=f"tsh{l}", name=f"tsh{l}")
                         for l in (1, 2)]
            tab_full = [dpool.tile([NPAD, H], bf16, addr_space="Shared",
                                   tag=f"tfl{l}", name=f"tfl{l}") for l in (1, 2)]
            ar_in = dpool.tile([P + 1, G], f32, tag="ar_in", name="ar_in")
            ar_out = dpool.tile([P + 1, G], f32, addr_space="Shared",
                                tag="ar_out", name="ar_out")

            # ---- constants ---------------------------------------------
            idxlo_sb = cpool.tile([P, BPC * FW], mybir.dt.int16)
            nc.sync.dma_start(out=idxlo_sb[:], in_=idxlo[:, :])
            idxhi_sb = cpool.tile([P, BPC * FW], mybir.dt.int16)
            nc.sync.dma_start(out=idxhi_sb[:], in_=idxhi[:, :])
            dst_sb = cpool.tile([P, BPC * CPB], f32)
            nc.sync.dma_start(out=dst_sb[:], in_=dstrel[:, :])
            iota_sb = cpool.tile([P, CPB * BS], f32)
            nc.sync.dma_start(out=iota_sb[:], in_=iota_rep[:, :])
            iota512_sb = cpool.tile([P, G], f32)
            nc.sync.dma_start(out=iota512_sb[:], in_=iota512[:, :])
            bat_sb = cpool.tile([P, BPC], f32)
            nc.sync.dma_start(out=bat_sb[:], in_=batf[:, :])
            W1_sb = cpool.tile([P, H], bf16)
            nc.gpsimd.dma_start(out=W1_sb[:], in_=W1f[:, :])   # SWDGE cast
            W2_sb = cpool.tile([P, H], bf16)
            nc.gpsimd.dma_start(out=W2_sb[:], in_=W2f[:, :])
            Wl_sb = cpool.tile([H, C], f32)
            nc.sync.dma_start(out=Wl_sb[:], in_=Wlf[:, :])
            b1_sb = cpool.tile([P, H], f32)
            nc.sync.dma_start(out=b1_sb[:], in_=b1r[:, :])
            b2_sb = cpool.tile([P, H], f32)
            nc.sync.dma_start(out=b2_sb[:], in_=b2r[:, :])
            bl_sb = cpool.tile([C, 1], f32)
            nc.sync.dma_start(out=bl_sb[:], in_=blc[:, :])
            ident = cpool.tile([P, P], f32)
            make_identity(nc, ident[:])
            ones_row = cpool.tile([1, P], f32)
            nc.vector.memset(ones_row[:], 1.0)
            ones_col = cpool.tile([P, 1], bf16)
            nc.vector.memset(ones_col[:], 1.0)

            deg_sb = cpool.tile([P, BPC], f32)
            nc.sync.dma_start(out=deg_sb[:], in_=degc[:, :])
            rec_sb = cpool.tile([P, BPC], f32)
            nc.vector.reciprocal(out=rec_sb[:], in_=deg_sb[:])
            dis_sb = cpool.tile([P, BPC], f32)
            nc.scalar.sqrt(out=dis_sb[:], in_=rec_sb[:])

            h1_sb = ppool.tile([P, BPC * H], f32, tag="h1")
            h2_sb = ppool.tile([P, BPC * H], bf16, tag="h2")

            def build_table(l, W_sb, h_src, pspool):
                for t in range(BPC):
                    if l == 0:
                        xf = wpool.tile([P, BS], f32, tag="xf")
                        nc.sync.dma_start(out=xf[:], in_=xt[t, :, :])
                        lhsT = wpool.tile([P, BS], bf16, tag="xT")
                        nc.vector.tensor_copy(out=lhsT[:], in_=xf[:])
                    else:
                        pst = pspool.tile([P, BS], f32, tag="pst")
                        nc.tensor.transpose(
                            out=pst[:], in_=h_src[:BS, t * H:(t + 1) * H],
                            identity=ident[:BS, :BS])
                        lhsT = wpool.tile([P, BS], bf16, tag="hT")
                        nc.vector.tensor_copy(out=lhsT[:], in_=pst[:])
                    psm = pspool.tile([BS, H], f32, tag="psm")
                    nc.tensor.matmul(out=psm[:], lhsT=lhsT[:], rhs=W_sb[:],
                                     start=True, stop=True)
                    tt = wpool.tile([BS, H], bf16, tag="tt")
                    nc.scalar.activation(
                        out=tt[:], in_=psm[:],
                        func=mybir.ActivationFunctionType.Copy,
                        scale=dis_sb[:BS, t:t + 1])
                    nc.sync.dma_start(out=tab_shard[l][t * BS:(t + 1) * BS, :],
                                      in_=tt[:])
                nc.gpsimd.collective_compute(
                    "AllGather", mybir.AluOpType.bypass, replica_groups=RG,
                    ins=[tab_shard[l][:, :]], outs=[tab_full[l][:, :]])

            def scatter_layer(l, b_sb, out_act, pspool, pool_ps=None):
                tf = tab_full[l]
                if pool_ps is not None:
                    ps_pool, ps_cnt = pool_ps
                for b in range(BPC):
                    gt = gpool.tile([P, SPB], bf16, tag="gt")
                    gt3 = gt[:].rearrange("p (c e) -> p c e", e=P)
                    nc.gpsimd.dma_gather(
                        out_ap=gt3[:, 0:CH, :], in_ap=tf[0:NHALF, :],
                        idxs_ap=idxlo_sb[:, b * FW:(b + 1) * FW],
                        num_idxs=CH * P, num_idxs_reg=CH * P, elem_size=H,
                        queue_num=(2 * b) % NQ)
                    nc.gpsimd.dma_gather(
                        out_ap=gt3[:, CH:CPB, :], in_ap=tf[NHALF:NPAD, :],
                        idxs_ap=idxhi_sb[:, b * FW:(b + 1) * FW],
                        num_idxs=CH * P, num_idxs_reg=CH * P, elem_size=H,
                        queue_num=(2 * b + 1) % NQ)
                    oh = ohpool.tile([P, CPB * BS], bf16, tag="oh")
                    nc.vector.tensor_tensor(
                        out=oh[:].rearrange("p (c e) -> p c e", e=BS),
                        in0=dst_sb[:, b * CPB:(b + 1) * CPB][:, :, None]
                            .to_broadcast([P, CPB, BS]),
                        in1=iota_sb[:].rearrange("p (c e) -> p c e", e=BS),
                        op=mybir.AluOpType.is_equal)
                    agg = pspool.tile([BS, H], f32, tag="agg")
                    for c in range(CPB):
                        nc.tensor.matmul(out=agg[:],
                                         lhsT=oh[:, c * BS:(c + 1) * BS],
                                         rhs=gt[:, c * P:(c + 1) * P],
                                         start=(c == 0), stop=(c == CPB - 1))
                    tw = eppool.tile([BS, H], bf16, tag="tw")
                    nc.sync.dma_start(out=tw[:],
                                      in_=tab_shard[l][b * BS:(b + 1) * BS, :])
                    u1 = eppool.tile([BS, H], f32, tag="u1")
                    nc.vector.tensor_tensor(
                        out=u1[:], in0=agg[:], in1=tw[:],
                        op=mybir.AluOpType.add)
                    u2 = eppool.tile([BS, H], f32, tag="u2")
                    nc.vector.tensor_scalar(
                        out=u2[:], in0=u1[:], scalar1=dis_sb[:BS, b:b + 1],
                        scalar2=None, op0=mybir.AluOpType.mult)
                    u3 = eppool.tile([BS, H], f32, tag="u3")
                    nc.vector.tensor_tensor(out=u3[:], in0=u2[:], in1=b_sb[:BS, :],
                                            op=mybir.AluOpType.add)
                    nc.scalar.activation(
                        out=out_act[:BS, b * H:(b + 1) * H], in_=u3[:],
                        func=mybir.ActivationFunctionType.Relu)
                    if pool_ps is not None:
                        oh5 = ohpool.tile([P, G], bf16, tag="oh5")
                        nc.vector.tensor_tensor(
                            out=oh5[:],
                            in0=bat_sb[:, b:b + 1].to_broadcast([P, G]),
                            in1=iota512_sb[:],
                            op=mybir.AluOpType.is_equal)
                        nc.tensor.matmul(out=ps_pool[:],
                                         lhsT=out_act[:BS, b * H:(b + 1) * H],
                                         rhs=oh5[:BS, :],
                                         start=(b == 0), stop=(b == BPC - 1))
                        nc.tensor.matmul(out=ps_cnt[:], lhsT=ones_col[:BS, :],
                                         rhs=oh5[:BS, :],
                                         start=(b == 0), stop=(b == BPC - 1))

            with tc.tile_pool(name="psAD", bufs=2, space="PSUM") as pspool, \
                 tc.tile_pool(name="psPool", bufs=1, space="PSUM") as plpool:
                ps_pool = plpool.tile([P, G], f32, tag="pool")
                ps_cnt = plpool.tile([1, G], f32, tag="cnt")
                build_table(0, W1_sb, None, pspool)
                scatter_layer(0, b1_sb, h1_sb, pspool)
                build_table(1, W2_sb, h1_sb, pspool)
                scatter_layer(1, b2_sb, h2_sb, pspool, (ps_pool, ps_cnt))

                # ---- pooling tail ------------------------------------
                sums_sb = ppool.tile([P, G], f32, tag="sums")
                nc.vector.tensor_copy(out=sums_sb[:], in_=ps_pool[:])
                cnt_sb = ppool.tile([1, G], f32, tag="cntsb")
                nc.vector.tensor_copy(out=cnt_sb[:], in_=ps_cnt[:])
                nc.sync.dma_start(out=ar_in[0:P, :], in_=sums_sb[:])
                nc.sync.dma_start(out=ar_in[P:P + 1, :], in_=cnt_sb[:])
            nc.gpsimd.collective_compute(
                "AllReduce", mybir.AluOpType.add, replica_groups=RG,
                ins=[ar_in[:, :]], outs=[ar_out[:, :]])
            psE = tc.tile_pool(name="psE", bufs=1, space="PSUM")
            pspool = psE.__enter__()
            sums2 = ppool.tile([P, G], f32, tag="sums2")
            nc.sync.dma_start(out=sums2[:], in_=ar_out[0:P, :])
            cnt2 = ppool.tile([1, G], f32, tag="cnt2")
            nc.sync.dma_start(out=cnt2[:], in_=ar_out[P:P + 1, :])
            cnt3 = ppool.tile([1, G], f32, tag="cnt3")
            nc.vector.tensor_scalar(out=cnt3[:], in0=cnt2[:], scalar1=1.0,
                                    scalar2=None, op0=mybir.AluOpType.max)
            rec = ppool.tile([1, G], f32, tag="rec")
            nc.vector.reciprocal(out=rec[:], in_=cnt3[:])
            ps_rb = pspool.tile([P, G], f32, tag="rb")
            nc.tensor.matmul(out=ps_rb[:], lhsT=ones_row[:], rhs=rec[:],
                             start=True, stop=True)
            means = ppool.tile([P, G], f32, tag="means")
            nc.vector.tensor_tensor(out=means[:], in0=sums2[:], in1=ps_rb[:],
                                    op=mybir.AluOpType.mult)
            ps_out = pspool.tile([C, G], f32, tag="out")
            nc.tensor.matmul(out=ps_out[:], lhsT=Wl_sb[:], rhs=means[:],
                             start=True, stop=True)
            outT = ppool.tile([C, G], f32, tag="outT")
            nc.scalar.activation(out=outT[:], in_=ps_out[:],
                                 func=mybir.ActivationFunctionType.Identity,
                                 bias=bl_sb[:, 0:1])
            for g in range(G // P):
                ps_tr = pspool.tile([P, C], f32, tag="tr")
                nc.tensor.transpose(out=ps_tr[:],
                                    in_=outT[:, g * P:(g + 1) * P],
                                    identity=ident[:C, :C])
                ot = eppool.tile([P, C], f32, tag="ot")
                nc.vector.tensor_copy(out=ot[:], in_=ps_tr[:])
                nc.sync.dma_start(out=out[g * P:(g + 1) * P, :], in_=ot[:])
            psE.__exit__(None, None, None)

    nc.compile()
    return nc


def kernel(x, edge_index, batch, W1, b1, W2, b2, Wl, bl, _trace=False):
    in_maps = _prep(x, edge_index, batch, W1, b1, W2, b2, Wl, bl)
    if "nc" not in _cache:
        _cache["nc"] = _build()
    nc = _cache["nc"]
    res = bass_utils.run_bass_kernel_spmd(
        nc, in_maps, core_ids=list(range(NCORES)), trace=_trace)
    kernel.last_result = res
    return res.results[0]["out"].astype(np.float32)

